# revision 52
# baseline (speedup 1.0000x reference)
"""Trainium2 Bass kernel for nn_Decoder (MLP -> inverse token embedding ->
overlap-add -> channel-merge conv), data-parallel over batch on 8 NeuronCores.

Self-contained: hardcodes shapes; host-side numpy folds everything after the
first Linear+ReLU into per-channel fused matrices G (W2 -> Winv -> overlap-add
normalization -> 3-tap channel conv), so the device pipeline is:

    xT[E,tok] (host pre-transposed, fp16) --matmul W1T--> h[Hc,tok] in PSUM
    --ACT/DVE relu+bias--> hT in SBUF --matmul G (accum over c,Hc)--> v[66,tok]
    --PE transpose--> vT[b,66] --strided adds (overlap-add)--> y[b,1056]

Sharding: batch 1024 -> 8 cores x 128.
"""

import numpy as np

import concourse.bacc as bacc
import concourse.mybir as mybir
from concourse.bass_utils import run_bass_kernel_spmd
from concourse.tile import TileContext

# problem shapes (hardcoded per contract)
B, C, T, E, H = 1024, 8, 32, 128, 256
SEG_LEN, SIG_LEN, NUM_SEG, STEP = 64, 1056, 32, 32
N_CORES = 8
BL = B // N_CORES          # local batch per core = 128
HC = H // 128              # H chunks = 2
TC = 8                     # t-chunks
TL = T // TC               # t per chunk = 4
CW = TL * 128              # columns per (c, t-chunk) = 512
FD = mybir.dt.float32
FR = mybir.dt.float32r   # fp32 storage, FP22 multiply: 4x faster PE
FH = mybir.dt.float16

_CACHE = {}


def _host_prep(W1, b1, W2, b2, Winv, binv, Wconv, bconv):
    """Fold W2/Winv/normalization/conv into G [3var][C][H,66] and bias B[1056]."""
    counter = np.zeros(SIG_LEN, np.float64)
    for t in range(NUM_SEG):
        counter[t * STEP: t * STEP + SEG_LEN] += 1.0
    n = 1.0 / counter

    F = Winv.astype(np.float64) @ W2.astype(np.float64)          # [64, H]
    binv2 = Winv.astype(np.float64) @ b2.astype(np.float64) + binv.astype(np.float64)
    Wc = Wconv[0].astype(np.float64)                             # [C, 3]

    def n_of(var, s):
        if var == 0:
            return n[s]
        if var == 2:
            return n[992 + s]
        return 0.5

    G = np.zeros((3, C, H, 66), np.float64)
    for var in range(3):
        for c in range(C):
            for m_idx in range(66):
                for k in range(3):
                    s = m_idx + k - 2
                    if 0 <= s < SEG_LEN:
                        G[var, c, :, m_idx] += Wc[c, k] * n_of(var, s) * F[s, :]

    sig_b = np.zeros(SIG_LEN, np.float64)
    for t in range(NUM_SEG):
        sig_b[t * STEP: t * STEP + SEG_LEN] += binv2
    sig_b *= n
    Bvec = np.full(SIG_LEN, float(np.asarray(bconv).reshape(-1)[0]), np.float64)
    q = np.arange(SIG_LEN)
    for k in range(3):
        qq = q + k - 1
        valid = (qq >= 0) & (qq < SIG_LEN)
        for c in range(C):
            Bvec[valid] += Wc[c, k] * sig_b[qq[valid]]
    return G.astype(np.float32), Bvec.astype(np.float32)


def _g_col(hc, c, var):
    """Column offset of G slice (hc, c, var) inside g_sb [128, 2*8*3*66].
    c-major so the split g load lands channels in consumption order."""
    return ((c * HC + hc) * 3 + var) * 66


def _build_bass():
    nc = bacc.Bacc("TRN2")

    # host pre-transposed to [TC, E, C*TL*BL] fp16: per t-chunk one contiguous
    # [128, 4096] block, columns ordered (c, tl, b)
    x = nc.dram_tensor("x", [TC, E, C * CW], FH, kind="ExternalInput")
    w1t = nc.dram_tensor("w1t", [E, H], FH, kind="ExternalInput")
    b1c = nc.dram_tensor("b1c", [128, HC], FD, kind="ExternalInput")
    g = nc.dram_tensor("g", [128, HC * C * 3 * 66], FH, kind="ExternalInput")
    bvec = nc.dram_tensor("bvec", [1, SIG_LEN], FD, kind="ExternalInput")
    ident = nc.dram_tensor("ident", [128, 128], FH, kind="ExternalInput")
    y = nc.dram_tensor("y", [BL, SIG_LEN], FD, kind="ExternalOutput")

    with TileContext(nc) as tc:
        with (
            tc.tile_pool(name="consts", bufs=1) as consts,
            tc.tile_pool(name="xin", bufs=2) as xin_pool,
            tc.tile_pool(name="ht", bufs=2) as ht_pool,
            tc.tile_pool(name="vsb", bufs=3) as vsb_pool,
            tc.tile_pool(name="big", bufs=1) as big_pool,
            tc.tile_pool(name="pe_out", bufs=1, space="PSUM") as peout_pool,
            tc.tile_pool(name="h_ps", bufs=4, space="PSUM") as hps_pool,
            tc.tile_pool(name="v_ps", bufs=3, space="PSUM") as vps_pool,
        ):
            w1t_sb = consts.tile([E, H], FH)
            b1c_sb = consts.tile([128, HC], FD)
            g_sb = consts.tile([128, HC * C * 3 * 66], FH)
            ident_sb = consts.tile([128, 128], FH)
            warm_a = consts.tile([128, 128], FH)
            warm_b = consts.tile([128, 384], FH)
            bvec_sb = consts.tile([1, SIG_LEN], FD)
            brep_sb = big_pool.tile([BL, SIG_LEN], FD)

            V_sb = big_pool.tile([BL, T * 66], FD)      # v transposed: [b, t*66+m]
            y_sb = big_pool.tile([BL, SIG_LEN], FD)

            xin_tiles = {}

            def emit_x_load(tcix, split_first=False):
                xt = xin_pool.tile([E, C * CW], FH, tag="xin")
                if split_first:
                    # first chunk in 4 pieces: more DMA-queue share against
                    # the const loads, and c0-c1 land early so MLP1 starts.
                    # The first piece dispatches from the Scalar HWDGE queue,
                    # which exits the preamble ~1us before Sync and is
                    # otherwise idle until the first relu drain.
                    nc.scalar.dma_start(out=xt[:, 0:2 * CW],
                                        in_=x[tcix, :, 0:2 * CW])
                    xin_tiles[tcix] = (xt, True)
                    return
                if tcix == 1:
                    # pieces interleaved with the g loads (see head order)
                    g_qtr = HC * 3 * 66 * 2
                    nc.sync.dma_start(out=xt[:, 0:2 * CW],
                                      in_=x[tcix, :, 0:2 * CW])
                    nc.sync.dma_start(out=g_sb[:, 0:g_qtr], in_=g[:, 0:g_qtr])
                    nc.sync.dma_start(out=xt[:, 2 * CW:4 * CW],
                                      in_=x[tcix, :, 2 * CW:4 * CW])
                    nc.sync.dma_start(out=g_sb[:, g_qtr:2 * g_qtr],
                                      in_=g[:, g_qtr:2 * g_qtr])
                    xin_tiles[tcix] = (xt, True)
                    return
                nc.sync.dma_start(out=xt[:], in_=x[tcix, :, :])
                xin_tiles[tcix] = (xt, False)

            def emit_x_load_second_half(tcix):
                xt, _ = xin_tiles[tcix]
                nc.sync.dma_start(out=xt[:, 4 * CW:C * CW],
                                  in_=x[tcix, :, 4 * CW:C * CW])

            def emit_x_load_rest(tcix):
                xt, _ = xin_tiles[tcix]
                for k in range(1, 4):
                    nc.sync.dma_start(
                        out=xt[:, 2 * k * CW:2 * (k + 1) * CW],
                        in_=x[tcix, :, 2 * k * CW:2 * (k + 1) * CW])

            # PE warm-up: the HAM clock gate releases only after ~3.4us of
            # sustained PE activity; burn garbage matmuls during the DMA head
            # so real matmuls start at 2.4 GHz. Reuses an h_ps buffer (WAW on
            # the in-order PE queue, so no stall).
            def emit_warmup():
                warm_ps = hps_pool.tile([128, CW], FD, tag="h_ps",
                                        name="warm_ps")
                nc.gpsimd.memset(warm_a[:], 1.0)
                nc.gpsimd.memset(warm_b[:], 1.0)
                # trigger ACT_TABLE_LOAD now, not at the first real relu
                nc.scalar.activation(
                    warm_a[:], warm_a[:],
                    mybir.ActivationFunctionType.Relu, scale=1.0)
                for _ in range(11):
                    nc.tensor.matmul(
                        warm_ps[:, 0:384], warm_b[:, 0:128], warm_b[:],
                        start=True, stop=True)

            # software pipeline: fused stage runs one t-chunk behind MLP1
            ht_tiles = {}

            # greedy ACT/DVE load balancer for PSUM->SBUF copies and relus
            eng_busy = {"act": 0.0, "dve": 0.0}

            def pick_engine(fd):
                ca = (352 + fd) / 1.2
                cd = (151 + fd) / 0.96
                if eng_busy["act"] + ca <= eng_busy["dve"] + cd:
                    eng_busy["act"] += ca
                    return "act"
                eng_busy["dve"] += cd
                return "dve"

            def bal_copy(out, in_, fd):
                if pick_engine(fd) == "act":
                    nc.scalar.copy(out=out, in_=in_)
                else:
                    nc.vector.tensor_copy(out=out, in_=in_)

            def chunk_ranges(tcix):
                # column ranges with uniform G variant; cols = tl*128 + b
                if tcix == 0:
                    return [(0, 128, 0), (128, CW, 1)]        # t=0 -> var 0
                if tcix == TC - 1:
                    return [(0, 384, 1), (384, CW, 2)]        # t=31 -> var 2
                return [(0, CW, 1)]

            def emit_mlp1(tcix, c):
                """MLP1 for one channel: 2 matmuls (one per hc) into 1-bank
                PSUM tiles; bufs=4 gives two channels of pipeline depth so
                the drains never stall the PE."""
                ht = ht_tiles[tcix]
                xt, _ = xin_tiles[tcix]
                cp, half = divmod(c, 2)
                h_list = []
                for hc in range(HC):
                    h_ps = hps_pool.tile([128, CW], FD, tag="h_ps",
                                         name=f"h_ps_{tcix}_{c}_{hc}")
                    nc.tensor.matmul(
                        h_ps[:],
                        w1t_sb[:, hc * 128:(hc + 1) * 128],
                        xt[:, c * CW:(c + 1) * CW],
                        start=True, stop=True,
                    )
                    h_list.append(h_ps)
                for hc in range(HC):
                    dst = ht[(cp, hc)][:, half * CW:(half + 1) * CW]
                    src = h_list[hc][:]
                    if pick_engine(CW) == "act":
                        nc.scalar.activation(
                            dst, src,
                            mybir.ActivationFunctionType.Relu,
                            bias=b1c_sb[:, hc:hc + 1], scale=1.0,
                        )
                    else:
                        nc.vector.tensor_scalar(
                            dst, src,
                            b1c_sb[:, hc:hc + 1], 0.0,
                            mybir.AluOpType.add, mybir.AluOpType.max,
                        )

            def emit_fused(tcix, v_tiles, c):
                """fused G matmuls for channel c accumulating into v_tiles."""
                ht = ht_tiles[tcix]
                cp, half = divmod(c, 2)
                for (lo, hi, var, v_ps) in v_tiles:
                    for hc in range(HC):
                        i = c * HC + hc
                        nc.tensor.matmul(
                            v_ps[:, lo:hi],
                            g_sb[:, _g_col(hc, c, var):_g_col(hc, c, var) + 66],
                            ht[(cp, hc)][:, half * CW + lo:half * CW + hi],
                            start=(i == 0), stop=(i == C * HC - 1),
                        )

            def emit_vtrans(tcix, v_tiles):
                """copy v psum -> sbuf, PE-transpose per t into one PSUM tile,
                single merged copy into V_sb."""
                del ht_tiles[tcix]
                v_sb = vsb_pool.tile([66, CW], FH, tag="v_sb")
                for (lo, hi, var, v_ps) in v_tiles:
                    bal_copy(v_sb[:, lo:hi], v_ps[:, lo:hi], hi - lo)
                vt_ps = peout_pool.tile([128, 264], FH, tag="pe_out")
                for tl in range(TL):
                    nc.tensor.transpose(
                        vt_ps[:, tl * 66:(tl + 1) * 66],
                        v_sb[:, tl * 128:(tl + 1) * 128],
                        ident_sb[0:66, 0:66],
                    )
                bal_copy(V_sb[:, tcix * 264:(tcix + 1) * 264], vt_ps[:], 264)

            # overlap-add assembly in rounds (per watermark) so it overlaps
            # with later chunks instead of serializing at the end
            V3 = V_sb[:].rearrange("b (t m) -> b t m", m=66)
            Y3 = y_sb[:].rearrange("b (j r) -> b j r", r=32)
            B3 = brep_sb[:].rearrange("b (j r) -> b j r", r=32)

            def emit_y_assembly(j_lo, j_hi, eng):
                """Assemble y blocks j in [j_lo, j_hi); requires V[t] for
                t <= j_hi (uses t=j+1 for the r=31 edge)."""
                jm = min(j_hi, 32)      # main1 defined for j<=31
                if jm > j_lo:
                    eng.tensor_add(
                        out=Y3[:, j_lo:jm, :], in0=V3[:, j_lo:jm, 1:33],
                        in1=B3[:, j_lo:jm, :])
                if j_hi == 33:          # last block: bias only here
                    eng.tensor_copy(
                        out=y_sb[:, 1024:1056], in_=brep_sb[:, 1024:1056])
                lo = max(1, j_lo)
                if j_hi > lo:           # += v[:, j-1, r+33]
                    eng.tensor_add(
                        out=Y3[:, lo:j_hi, :], in0=Y3[:, lo:j_hi, :],
                        in1=V3[:, lo - 1:j_hi - 1, 33:65])
                lo = max(2, j_lo)
                if j_hi > lo:           # r=0: += v[:, j-2, 65]
                    eng.tensor_add(
                        out=Y3[:, lo:j_hi, 0], in0=Y3[:, lo:j_hi, 0],
                        in1=V3[:, lo - 2:j_hi - 2, 65])
                hi = min(j_hi, 31)
                if hi > j_lo:           # r=31: += v[:, j+1, 0]
                    eng.tensor_add(
                        out=Y3[:, j_lo:hi, 31], in0=Y3[:, j_lo:hi, 31],
                        in1=V3[:, j_lo + 1:hi + 1, 0])

            # after vtrans(i) V[t] is final for t <= 4i+3, so y blocks
            # j < min(4i+3, 33) can assemble (block j reads up to t=j+1)
            y_wm = [0]

            def emit_rounds(i):
                if i < TC - 1:
                    j_hi = min(4 * i + 3, 33)
                    if j_hi > y_wm[0]:
                        emit_y_assembly(y_wm[0], j_hi, nc.gpsimd)
                        y_wm[0] = j_hi
                else:
                    # final round: split across gpsimd + vector (independent
                    # j ranges) to shorten the tail, each half stored as soon
                    # as it is assembled
                    mid = (y_wm[0] + 33 + 1) // 2
                    emit_y_assembly(y_wm[0], mid, nc.gpsimd)
                    nc.gpsimd.dma_start(out=y[:, 864:32 * mid],
                                        in_=y_sb[:, 864:32 * mid])
                    emit_y_assembly(mid, 33, nc.vector)
                    y_wm[0] = 33
                # progressive stores once column ranges are final (on the
                # gpsimd queue so their waits never block Sync x-dispatches)
                if y_wm[0] >= 15 and not store_done[0]:
                    nc.gpsimd.dma_start(out=y[:, 0:480], in_=y_sb[:, 0:480])
                    store_done[0] = True
                if y_wm[0] >= 27 and not store_done[1]:
                    nc.gpsimd.dma_start(out=y[:, 480:864], in_=y_sb[:, 480:864])
                    store_done[1] = True

            store_done = [False, False]

            prev = None          # (tcix, v_tiles) of the chunk awaiting fused
            emit_x_load(0, split_first=True)
            emit_warmup()
            nc.sync.dma_start(out=w1t_sb[:], in_=w1t[:])
            nc.sync.dma_start(out=b1c_sb[:], in_=b1c[:])
            emit_x_load_rest(0)
            emit_x_load(1)
            emit_x_load_second_half(1)
            g_half = HC * C * 3 * 66 // 2
            nc.sync.dma_start(out=g_sb[:, g_half:], in_=g[:, g_half:])
            nc.sync.dma_start(out=ident_sb[:], in_=ident[:])
            nc.sync.dma_start(out=bvec_sb[:], in_=bvec[:])
            # bias row is tiny: broadcast on-device instead of loading 128
            # replicated rows over the contended early DMA window
            nc.gpsimd.partition_broadcast(brep_sb[:], bvec_sb[:])
            for tcix in range(TC):
                ht_tiles[tcix] = {
                    (cp, hc): ht_pool.tile(
                        [128, 2 * CW], FH,
                        tag=f"ht{hc}_{cp}", name=f"ht_{tcix}_{hc}_{cp}")
                    for cp in range(C // 2) for hc in range(HC)}
                if tcix + 2 < TC:
                    emit_x_load(tcix + 2)
                # interleave: MLP1(tcix, cp) with fused(tcix-1, c) so PE
                # always has matmul work while relu copies drain PSUM
                for c in range(C):
                    emit_mlp1(tcix, c)
                    if prev is not None:
                        emit_fused(prev[0], prev[1], c)
                if prev is not None:
                    emit_vtrans(prev[0], prev[1])
                    emit_rounds(prev[0])
                del xin_tiles[tcix]
                v_tiles = [
                    (lo, hi, var, vps_pool.tile(
                        [66, CW], FD, tag="v_ps", name=f"v_ps_{tcix}_{lo}"))
                    for (lo, hi, var) in chunk_ranges(tcix)]
                prev = (tcix, v_tiles)
            for c in range(C):
                emit_fused(prev[0], prev[1], c)
            emit_vtrans(prev[0], prev[1])
            emit_rounds(prev[0])

            mid_col = 32 * ((27 + 33 + 1) // 2)
            nc.sync.dma_start(out=y[:, mid_col:SIG_LEN],
                              in_=y_sb[:, mid_col:SIG_LEN])

    nc.finalize()
    return nc


def make_in_maps(inputs):
    """Per-core input maps (shared by kernel(), sim checks, and bench)."""
    x = np.asarray(inputs["encoder_output"], dtype=np.float32)
    W1 = np.asarray(inputs["W1"], np.float32)
    b1 = np.asarray(inputs["b1"], np.float32)

    G, Bvec = _host_prep(
        inputs["W1"], inputs["b1"], inputs["W2"], inputs["b2"],
        inputs["Winv"], inputs["binv"], inputs["Wconv"], inputs["bconv"])

    # pack G -> [128, HC*C*3*66]: g_sb[p, _g_col(hc,c,var)+m] = G[var, c, hc*128+p, m]
    g_pack = np.zeros((128, HC * C * 3 * 66), np.float32)
    for hc in range(HC):
        for c in range(C):
            for var in range(3):
                col = _g_col(hc, c, var)
                g_pack[:, col:col + 66] = G[var, c, hc * 128:(hc + 1) * 128, :]

    w1t = np.ascontiguousarray(W1.T).astype(np.float16)     # [E, H]
    g_pack = g_pack.astype(np.float16)
    b1c = np.ascontiguousarray(b1.reshape(HC, 128).T)       # [128, HC]
    bvec = np.ascontiguousarray(Bvec.reshape(1, SIG_LEN))
    ident = np.eye(128, dtype=np.float16)

    # [B,C,T,E] -> per-core [TC, E, (c, tl, b)] fp16 (device reads xT directly)
    xs = x.reshape(N_CORES, BL, C, TC, TL, E).transpose(0, 3, 5, 2, 4, 1)
    xs = np.ascontiguousarray(xs.astype(np.float16)).reshape(
        N_CORES, TC, E, C * CW)
    return [
        {
            "x": xs[i],
            "w1t": w1t, "b1c": b1c, "g": g_pack,
            "bvec": bvec, "ident": ident,
        }
        for i in range(N_CORES)
    ]


def kernel(**inputs) -> np.ndarray:
    if "nc" not in _CACHE:
        _CACHE["nc"] = _build_bass()
    nc = _CACHE["nc"]

    in_maps = make_in_maps(inputs)
    try:
        res = run_bass_kernel_spmd(nc, in_maps, core_ids=list(range(N_CORES)))
    except ModuleNotFoundError:
        # BASS_TRACE was set but the axon NTFF profile hook module is absent
        # in this environment; rerun without tracing.
        import os
        os.environ["BASS_NEVER_TRACE"] = "1"
        res = run_bass_kernel_spmd(nc, in_maps, core_ids=list(range(N_CORES)))
    _CACHE["last_result"] = res
    y = np.concatenate([r["y"] for r in res.results], axis=0)   # [B, 1056]
    return y.reshape(B, 1, SIG_LEN).astype(np.float32)


if __name__ == "__main__":
    rng = np.random.default_rng(0)
    ins = {
        "encoder_output": rng.standard_normal((B, C, T, E), dtype=np.float32),
        "W1": rng.standard_normal((H, E), dtype=np.float32) / np.sqrt(E),
        "b1": rng.standard_normal((H,), dtype=np.float32) / np.sqrt(E),
        "W2": rng.standard_normal((E, H), dtype=np.float32) / np.sqrt(H),
        "b2": rng.standard_normal((E,), dtype=np.float32) / np.sqrt(H),
        "Winv": rng.standard_normal((SEG_LEN, E), dtype=np.float32) / np.sqrt(E),
        "binv": rng.standard_normal((SEG_LEN,), dtype=np.float32) / np.sqrt(E),
        "Wconv": rng.standard_normal((1, C, 3), dtype=np.float32) / np.sqrt(C * 3),
        "bconv": rng.standard_normal((1,), dtype=np.float32) / np.sqrt(C * 3),
    }
    out = kernel(**ins)
    print("kernel output", out.shape, out.dtype)


# revision 55
# speedup vs baseline: 1.1278x; 1.1278x over previous
"""Trainium2 Bass kernel for nn_Decoder (MLP -> inverse token embedding ->
overlap-add -> channel-merge conv), data-parallel over batch on 8 NeuronCores.

Self-contained: hardcodes shapes; host-side numpy folds everything after the
first Linear+ReLU into per-channel fused matrices G (W2 -> Winv -> overlap-add
normalization -> 3-tap channel conv), so the device pipeline is:

    xT[E,tok] (host pre-transposed, fp16) --matmul W1T--> h[Hc,tok] in PSUM
    --ACT/DVE relu+bias--> hT in SBUF --matmul G (accum over c,Hc)--> v[66,tok]
    --PE transpose--> vT[b,66] --strided adds (overlap-add)--> y[b,1056]

Sharding: batch 1024 -> 8 cores x 128.
"""

import numpy as np

import concourse.bacc as bacc
import concourse.mybir as mybir
from concourse.bass_utils import run_bass_kernel_spmd
from concourse.tile import TileContext

# problem shapes (hardcoded per contract)
B, C, T, E, H = 1024, 8, 32, 128, 256
SEG_LEN, SIG_LEN, NUM_SEG, STEP = 64, 1056, 32, 32
N_CORES = 8
BL = B // N_CORES          # local batch per core = 128
HC = H // 128              # H chunks = 2
TC = 8                     # t-chunks
TL = T // TC               # t per chunk = 4
CW = TL * 128              # columns per (c, t-chunk) = 512
FD = mybir.dt.float32
FR = mybir.dt.float32r   # fp32 storage, FP22 multiply: 4x faster PE
FH = mybir.dt.float16

_CACHE = {}


def _host_prep(W1, b1, W2, b2, Winv, binv, Wconv, bconv):
    """Fold W2/Winv/normalization/conv into G [3var][C][H,66] and bias B[1056]."""
    counter = np.zeros(SIG_LEN, np.float64)
    for t in range(NUM_SEG):
        counter[t * STEP: t * STEP + SEG_LEN] += 1.0
    n = 1.0 / counter

    F = Winv.astype(np.float64) @ W2.astype(np.float64)          # [64, H]
    binv2 = Winv.astype(np.float64) @ b2.astype(np.float64) + binv.astype(np.float64)
    Wc = Wconv[0].astype(np.float64)                             # [C, 3]

    def n_of(var, s):
        if var == 0:
            return n[s]
        if var == 2:
            return n[992 + s]
        return 0.5

    G = np.zeros((3, C, H, 66), np.float64)
    for var in range(3):
        for c in range(C):
            for m_idx in range(66):
                for k in range(3):
                    s = m_idx + k - 2
                    if 0 <= s < SEG_LEN:
                        G[var, c, :, m_idx] += Wc[c, k] * n_of(var, s) * F[s, :]

    sig_b = np.zeros(SIG_LEN, np.float64)
    for t in range(NUM_SEG):
        sig_b[t * STEP: t * STEP + SEG_LEN] += binv2
    sig_b *= n
    Bvec = np.full(SIG_LEN, float(np.asarray(bconv).reshape(-1)[0]), np.float64)
    q = np.arange(SIG_LEN)
    for k in range(3):
        qq = q + k - 1
        valid = (qq >= 0) & (qq < SIG_LEN)
        for c in range(C):
            Bvec[valid] += Wc[c, k] * sig_b[qq[valid]]
    return G.astype(np.float32), Bvec.astype(np.float32)


def _g_col(hc, c, var):
    """Column offset of G slice (hc, c, var) inside g_sb [128, 2*8*3*66].
    c-major so the split g load lands channels in consumption order."""
    return ((c * HC + hc) * 3 + var) * 66


def _build_bass():
    nc = bacc.Bacc("TRN2")

    # host pre-transposed to [TC, E, C*TL*BL] fp16: per t-chunk one contiguous
    # [128, 4096] block, columns ordered (c, tl, b)
    x = nc.dram_tensor("x", [TC, E, C * CW], FH, kind="ExternalInput")
    w1t = nc.dram_tensor("w1t", [E, H], FH, kind="ExternalInput")
    b1c = nc.dram_tensor("b1c", [128, HC], FD, kind="ExternalInput")
    g = nc.dram_tensor("g", [128, HC * C * 3 * 66], FH, kind="ExternalInput")
    bvec = nc.dram_tensor("bvec", [1, SIG_LEN], FD, kind="ExternalInput")
    ident = nc.dram_tensor("ident", [128, 128], FH, kind="ExternalInput")
    y = nc.dram_tensor("y", [BL, SIG_LEN], FD, kind="ExternalOutput")

    with TileContext(nc) as tc:
        with (
            tc.tile_pool(name="consts", bufs=1) as consts,
            tc.tile_pool(name="xin", bufs=2) as xin_pool,
            tc.tile_pool(name="ht", bufs=2) as ht_pool,
            tc.tile_pool(name="vsb", bufs=3) as vsb_pool,
            tc.tile_pool(name="big", bufs=1) as big_pool,
            tc.tile_pool(name="pe_out", bufs=1, space="PSUM") as peout_pool,
            tc.tile_pool(name="h_ps", bufs=4, space="PSUM") as hps_pool,
            tc.tile_pool(name="v_ps", bufs=3, space="PSUM") as vps_pool,
        ):
            w1t_sb = consts.tile([E, H], FH)
            b1c_sb = consts.tile([128, HC], FD)
            g_sb = consts.tile([128, HC * C * 3 * 66], FH)
            ident_sb = consts.tile([128, 128], FH)
            warm_a = consts.tile([128, 128], FH)
            warm_b = consts.tile([128, 384], FH)
            bvec_sb = consts.tile([1, SIG_LEN], FD)
            brep_sb = big_pool.tile([BL, SIG_LEN], FD)

            V_sb = big_pool.tile([BL, T * 66], FD)      # v transposed: [b, t*66+m]
            y_sb = big_pool.tile([BL, SIG_LEN], FD)

            xin_tiles = {}

            def emit_x_load(tcix, split_first=False):
                xt = xin_pool.tile([E, C * CW], FH, tag="xin")
                if split_first:
                    # first chunk in 4 pieces: more DMA-queue share against
                    # the const loads, and c0-c1 land early so MLP1 starts.
                    # The first piece dispatches from the Scalar HWDGE queue,
                    # which exits the preamble ~1us before Sync and is
                    # otherwise idle until the first relu drain.
                    nc.scalar.dma_start(out=xt[:, 0:2 * CW],
                                        in_=x[tcix, :, 0:2 * CW])
                    xin_tiles[tcix] = (xt, True)
                    return
                if tcix == 1:
                    # first quarter separately so tcix1's c0-c1 land sooner
                    nc.sync.dma_start(out=xt[:, 0:2 * CW],
                                      in_=x[tcix, :, 0:2 * CW])
                    nc.sync.dma_start(out=xt[:, 2 * CW:4 * CW],
                                      in_=x[tcix, :, 2 * CW:4 * CW])
                    xin_tiles[tcix] = (xt, True)
                    return
                nc.sync.dma_start(out=xt[:], in_=x[tcix, :, :])
                xin_tiles[tcix] = (xt, False)

            def emit_x_load_second_half(tcix):
                xt, _ = xin_tiles[tcix]
                nc.sync.dma_start(out=xt[:, 4 * CW:C * CW],
                                  in_=x[tcix, :, 4 * CW:C * CW])

            def emit_x_load_rest(tcix):
                xt, _ = xin_tiles[tcix]
                for k in range(1, 4):
                    nc.sync.dma_start(
                        out=xt[:, 2 * k * CW:2 * (k + 1) * CW],
                        in_=x[tcix, :, 2 * k * CW:2 * (k + 1) * CW])

            # PE warm-up: the HAM clock gate releases only after ~3.4us of
            # sustained PE activity; burn garbage matmuls during the DMA head
            # so real matmuls start at 2.4 GHz. Reuses an h_ps buffer (WAW on
            # the in-order PE queue, so no stall).
            def emit_warmup():
                warm_ps = hps_pool.tile([128, CW], FD, tag="h_ps",
                                        name="warm_ps")
                nc.gpsimd.memset(warm_a[:], 1.0)
                nc.gpsimd.memset(warm_b[:], 1.0)
                # trigger ACT_TABLE_LOAD now, not at the first real relu
                nc.scalar.activation(
                    warm_a[:], warm_a[:],
                    mybir.ActivationFunctionType.Relu, scale=1.0)
                for _ in range(11):
                    nc.tensor.matmul(
                        warm_ps[:, 0:384], warm_b[:, 0:128], warm_b[:],
                        start=True, stop=True)

            # software pipeline: fused stage runs one t-chunk behind MLP1
            ht_tiles = {}

            # greedy ACT/DVE load balancer for PSUM->SBUF copies and relus
            eng_busy = {"act": 0.0, "dve": 0.0}

            def pick_engine(fd):
                ca = (352 + fd) / 1.2
                cd = (151 + fd) / 0.96
                if eng_busy["act"] + ca <= eng_busy["dve"] + cd:
                    eng_busy["act"] += ca
                    return "act"
                eng_busy["dve"] += cd
                return "dve"

            def bal_copy(out, in_, fd):
                if pick_engine(fd) == "act":
                    nc.scalar.copy(out=out, in_=in_)
                else:
                    nc.vector.tensor_copy(out=out, in_=in_)

            def chunk_ranges(tcix):
                # column ranges with uniform G variant; cols = tl*128 + b
                if tcix == 0:
                    return [(0, 128, 0), (128, CW, 1)]        # t=0 -> var 0
                if tcix == TC - 1:
                    return [(0, 384, 1), (384, CW, 2)]        # t=31 -> var 2
                return [(0, CW, 1)]

            def emit_mlp1(tcix, c):
                """MLP1 for one channel: 2 matmuls (one per hc) into 1-bank
                PSUM tiles; bufs=4 gives two channels of pipeline depth so
                the drains never stall the PE."""
                ht = ht_tiles[tcix]
                xt, _ = xin_tiles[tcix]
                cp, half = divmod(c, 2)
                h_list = []
                for hc in range(HC):
                    h_ps = hps_pool.tile([128, CW], FD, tag="h_ps",
                                         name=f"h_ps_{tcix}_{c}_{hc}")
                    nc.tensor.matmul(
                        h_ps[:],
                        w1t_sb[:, hc * 128:(hc + 1) * 128],
                        xt[:, c * CW:(c + 1) * CW],
                        start=True, stop=True,
                    )
                    h_list.append(h_ps)
                for hc in range(HC):
                    dst = ht[(cp, hc)][:, half * CW:(half + 1) * CW]
                    src = h_list[hc][:]
                    if pick_engine(CW) == "act":
                        nc.scalar.activation(
                            dst, src,
                            mybir.ActivationFunctionType.Relu,
                            bias=b1c_sb[:, hc:hc + 1], scale=1.0,
                        )
                    else:
                        nc.vector.tensor_scalar(
                            dst, src,
                            b1c_sb[:, hc:hc + 1], 0.0,
                            mybir.AluOpType.add, mybir.AluOpType.max,
                        )

            def emit_fused(tcix, v_tiles, c):
                """fused G matmuls for channel c accumulating into v_tiles."""
                ht = ht_tiles[tcix]
                cp, half = divmod(c, 2)
                for (lo, hi, var, v_ps) in v_tiles:
                    for hc in range(HC):
                        i = c * HC + hc
                        nc.tensor.matmul(
                            v_ps[:, lo:hi],
                            g_sb[:, _g_col(hc, c, var):_g_col(hc, c, var) + 66],
                            ht[(cp, hc)][:, half * CW + lo:half * CW + hi],
                            start=(i == 0), stop=(i == C * HC - 1),
                        )

            def emit_vtrans(tcix, v_tiles):
                """copy v psum -> sbuf, PE-transpose per t into one PSUM tile,
                single merged copy into V_sb."""
                del ht_tiles[tcix]
                v_sb = vsb_pool.tile([66, CW], FH, tag="v_sb")
                for (lo, hi, var, v_ps) in v_tiles:
                    bal_copy(v_sb[:, lo:hi], v_ps[:, lo:hi], hi - lo)
                vt_ps = peout_pool.tile([128, 264], FH, tag="pe_out")
                for tl in range(TL):
                    nc.tensor.transpose(
                        vt_ps[:, tl * 66:(tl + 1) * 66],
                        v_sb[:, tl * 128:(tl + 1) * 128],
                        ident_sb[0:66, 0:66],
                    )
                bal_copy(V_sb[:, tcix * 264:(tcix + 1) * 264], vt_ps[:], 264)

            # overlap-add assembly in rounds (per watermark) so it overlaps
            # with later chunks instead of serializing at the end
            V3 = V_sb[:].rearrange("b (t m) -> b t m", m=66)
            Y3 = y_sb[:].rearrange("b (j r) -> b j r", r=32)
            B3 = brep_sb[:].rearrange("b (j r) -> b j r", r=32)

            def emit_y_assembly(j_lo, j_hi, eng):
                """Assemble y blocks j in [j_lo, j_hi); requires V[t] for
                t <= j_hi (uses t=j+1 for the r=31 edge)."""
                jm = min(j_hi, 32)      # main1 defined for j<=31
                if jm > j_lo:
                    eng.tensor_add(
                        out=Y3[:, j_lo:jm, :], in0=V3[:, j_lo:jm, 1:33],
                        in1=B3[:, j_lo:jm, :])
                if j_hi == 33:          # last block: bias only here
                    eng.tensor_copy(
                        out=y_sb[:, 1024:1056], in_=brep_sb[:, 1024:1056])
                lo = max(1, j_lo)
                if j_hi > lo:           # += v[:, j-1, r+33]
                    eng.tensor_add(
                        out=Y3[:, lo:j_hi, :], in0=Y3[:, lo:j_hi, :],
                        in1=V3[:, lo - 1:j_hi - 1, 33:65])
                lo = max(2, j_lo)
                if j_hi > lo:           # r=0: += v[:, j-2, 65]
                    eng.tensor_add(
                        out=Y3[:, lo:j_hi, 0], in0=Y3[:, lo:j_hi, 0],
                        in1=V3[:, lo - 2:j_hi - 2, 65])
                hi = min(j_hi, 31)
                if hi > j_lo:           # r=31: += v[:, j+1, 0]
                    eng.tensor_add(
                        out=Y3[:, j_lo:hi, 31], in0=Y3[:, j_lo:hi, 31],
                        in1=V3[:, j_lo + 1:hi + 1, 0])

            # after vtrans(i) V[t] is final for t <= 4i+3, so y blocks
            # j < min(4i+3, 33) can assemble (block j reads up to t=j+1)
            y_wm = [0]

            def emit_rounds(i):
                if i < TC - 1:
                    j_hi = min(4 * i + 3, 33)
                    if j_hi > y_wm[0]:
                        emit_y_assembly(y_wm[0], j_hi, nc.gpsimd)
                        y_wm[0] = j_hi
                else:
                    # final round: split across gpsimd + vector (independent
                    # j ranges) to shorten the tail, each half stored as soon
                    # as it is assembled
                    mid = (y_wm[0] + 33 + 1) // 2
                    emit_y_assembly(y_wm[0], mid, nc.gpsimd)
                    nc.gpsimd.dma_start(out=y[:, 864:32 * mid],
                                        in_=y_sb[:, 864:32 * mid])
                    emit_y_assembly(mid, 33, nc.vector)
                    y_wm[0] = 33
                # progressive stores once column ranges are final (on the
                # gpsimd queue so their waits never block Sync x-dispatches)
                if y_wm[0] >= 15 and not store_done[0]:
                    nc.gpsimd.dma_start(out=y[:, 0:480], in_=y_sb[:, 0:480])
                    store_done[0] = True
                if y_wm[0] >= 27 and not store_done[1]:
                    nc.gpsimd.dma_start(out=y[:, 480:864], in_=y_sb[:, 480:864])
                    store_done[1] = True

            store_done = [False, False]

            prev = None          # (tcix, v_tiles) of the chunk awaiting fused
            emit_x_load(0, split_first=True)
            emit_warmup()
            nc.sync.dma_start(out=w1t_sb[:], in_=w1t[:])
            nc.sync.dma_start(out=b1c_sb[:], in_=b1c[:])
            emit_x_load_rest(0)
            emit_x_load(1)
            g_half = HC * C * 3 * 66 // 2
            # g first half rides the otherwise-idle Scalar HWDGE queue: lands
            # sooner AND frees a Sync head slot so the x2+ prefetch
            # dispatches move earlier (the v13 lesson, in reverse)
            nc.scalar.dma_start(out=g_sb[:, 0:g_half], in_=g[:, 0:g_half])
            emit_x_load_second_half(1)
            nc.sync.dma_start(out=g_sb[:, g_half:], in_=g[:, g_half:])
            nc.sync.dma_start(out=ident_sb[:], in_=ident[:])
            nc.sync.dma_start(out=bvec_sb[:], in_=bvec[:])
            # bias row is tiny: broadcast on-device instead of loading 128
            # replicated rows over the contended early DMA window
            nc.gpsimd.partition_broadcast(brep_sb[:], bvec_sb[:])
            for tcix in range(TC):
                ht_tiles[tcix] = {
                    (cp, hc): ht_pool.tile(
                        [128, 2 * CW], FH,
                        tag=f"ht{hc}_{cp}", name=f"ht_{tcix}_{hc}_{cp}")
                    for cp in range(C // 2) for hc in range(HC)}
                if tcix + 2 < TC:
                    emit_x_load(tcix + 2)
                # interleave: MLP1(tcix, cp) with fused(tcix-1, c) so PE
                # always has matmul work while relu copies drain PSUM
                for c in range(C):
                    emit_mlp1(tcix, c)
                    if prev is not None:
                        emit_fused(prev[0], prev[1], c)
                if prev is not None:
                    emit_vtrans(prev[0], prev[1])
                    emit_rounds(prev[0])
                del xin_tiles[tcix]
                v_tiles = [
                    (lo, hi, var, vps_pool.tile(
                        [66, CW], FD, tag="v_ps", name=f"v_ps_{tcix}_{lo}"))
                    for (lo, hi, var) in chunk_ranges(tcix)]
                prev = (tcix, v_tiles)
            for c in range(C):
                emit_fused(prev[0], prev[1], c)
            emit_vtrans(prev[0], prev[1])
            emit_rounds(prev[0])

            mid_col = 32 * ((27 + 33 + 1) // 2)
            nc.sync.dma_start(out=y[:, mid_col:SIG_LEN],
                              in_=y_sb[:, mid_col:SIG_LEN])

    nc.finalize()
    return nc


def make_in_maps(inputs):
    """Per-core input maps (shared by kernel(), sim checks, and bench)."""
    x = np.asarray(inputs["encoder_output"], dtype=np.float32)
    W1 = np.asarray(inputs["W1"], np.float32)
    b1 = np.asarray(inputs["b1"], np.float32)

    G, Bvec = _host_prep(
        inputs["W1"], inputs["b1"], inputs["W2"], inputs["b2"],
        inputs["Winv"], inputs["binv"], inputs["Wconv"], inputs["bconv"])

    # pack G -> [128, HC*C*3*66]: g_sb[p, _g_col(hc,c,var)+m] = G[var, c, hc*128+p, m]
    g_pack = np.zeros((128, HC * C * 3 * 66), np.float32)
    for hc in range(HC):
        for c in range(C):
            for var in range(3):
                col = _g_col(hc, c, var)
                g_pack[:, col:col + 66] = G[var, c, hc * 128:(hc + 1) * 128, :]

    w1t = np.ascontiguousarray(W1.T).astype(np.float16)     # [E, H]
    g_pack = g_pack.astype(np.float16)
    b1c = np.ascontiguousarray(b1.reshape(HC, 128).T)       # [128, HC]
    bvec = np.ascontiguousarray(Bvec.reshape(1, SIG_LEN))
    ident = np.eye(128, dtype=np.float16)

    # [B,C,T,E] -> per-core [TC, E, (c, tl, b)] fp16 (device reads xT directly)
    xs = x.reshape(N_CORES, BL, C, TC, TL, E).transpose(0, 3, 5, 2, 4, 1)
    xs = np.ascontiguousarray(xs.astype(np.float16)).reshape(
        N_CORES, TC, E, C * CW)
    return [
        {
            "x": xs[i],
            "w1t": w1t, "b1c": b1c, "g": g_pack,
            "bvec": bvec, "ident": ident,
        }
        for i in range(N_CORES)
    ]


def kernel(**inputs) -> np.ndarray:
    if "nc" not in _CACHE:
        _CACHE["nc"] = _build_bass()
    nc = _CACHE["nc"]

    in_maps = make_in_maps(inputs)
    try:
        res = run_bass_kernel_spmd(nc, in_maps, core_ids=list(range(N_CORES)))
    except ModuleNotFoundError:
        # BASS_TRACE was set but the axon NTFF profile hook module is absent
        # in this environment; rerun without tracing.
        import os
        os.environ["BASS_NEVER_TRACE"] = "1"
        res = run_bass_kernel_spmd(nc, in_maps, core_ids=list(range(N_CORES)))
    _CACHE["last_result"] = res
    y = np.concatenate([r["y"] for r in res.results], axis=0)   # [B, 1056]
    return y.reshape(B, 1, SIG_LEN).astype(np.float32)


if __name__ == "__main__":
    rng = np.random.default_rng(0)
    ins = {
        "encoder_output": rng.standard_normal((B, C, T, E), dtype=np.float32),
        "W1": rng.standard_normal((H, E), dtype=np.float32) / np.sqrt(E),
        "b1": rng.standard_normal((H,), dtype=np.float32) / np.sqrt(E),
        "W2": rng.standard_normal((E, H), dtype=np.float32) / np.sqrt(H),
        "b2": rng.standard_normal((E,), dtype=np.float32) / np.sqrt(H),
        "Winv": rng.standard_normal((SEG_LEN, E), dtype=np.float32) / np.sqrt(E),
        "binv": rng.standard_normal((SEG_LEN,), dtype=np.float32) / np.sqrt(E),
        "Wconv": rng.standard_normal((1, C, 3), dtype=np.float32) / np.sqrt(C * 3),
        "bconv": rng.standard_normal((1,), dtype=np.float32) / np.sqrt(C * 3),
    }
    out = kernel(**ins)
    print("kernel output", out.shape, out.dtype)
